# revision 4
# baseline (speedup 1.0000x reference)
"""Josephson-junction SDE Euler-Maruyama kernel for 8 Trainium2 NeuronCores.

Per core (batch 2048 = 32 groups x 64 columns), skewed state
Y~_t = [phi1_{t+1}, phi2_{t+1}, v1_t, v2_t] laid out one 32-partition block per
component. Substituting phi_t = phi_{t+1} - dt*v_t makes the update linear in
the skewed state, so each step is two fp32 matmuls into one PSUM bank
(MM2: sin/noise terms via W_SZ; MM1: drift via W_A) plus a DVE
tensor_scalar_add eviction that adds the per-partition dt*i constants.

sin(phi) needs |arg| <~ 3.3 for the ACT LUT while phi reaches ~600, so the
sine input is range-reduced with a fractional-turns pipeline (y = phi/2pi;
u = y - round(y) via the 2^23 magic-number trick; sin = ACT Sin with
scale = 2pi and an exactly-representable bias 256*2pi):
  DVE : q = phi*ic + (2^23+256)   (rounds y+256 to an integer)
  POOL: p = phi*ic ;  n = q - 2^23 ;  w = p - n   (= frac(y) - 256)
  ACT : S = Sin(w*c + 256c)
The skewed state gives this chain two full steps of scheduling slack.

Everything is fully unrolled with static semaphore thresholds; noise streams
in and the trajectory streams out as ~1 MiB per-64-step block DMAs with
per-partition-contiguous DRAM layouts ([comp, g, t, j]).
"""

import math

import numpy as np

import concourse.bass as bass
import concourse.mybir as mybir
from concourse import bass_utils

F32 = mybir.dt.float32
A = mybir.AluOpType
N_CORES = 8
BATCH = 16384
BPC = BATCH // N_CORES  # 2048
G = 32  # partition groups per component
J = 64  # batch columns per step-slot
BS = 64  # steps per ring block
NBANK = 8
RR = 4  # reduction ring slots

IC = float(np.float32(1.0 / (2 * math.pi)))
C2PI = float(np.float32(2 * math.pi))
B1 = 8388864.0  # 2^23 + 256
B2 = 8388608.0  # 2^23

_CACHE = {}


def _build_program(nt):
    """Per-core bass program integrating nt steps (slots 0..nt)."""
    nslot = nt + 1
    nblk = (nslot + BS - 1) // BS
    nred = max(0, nt - 2)  # reduction chains: t = 0..nt-3 (slot t+1, S_{t+2})
    nc = bass.Bass()

    w_a_d = nc.dram_tensor("w_a", [128, 128], F32, kind="ExternalInput")
    w_sz_d = nc.dram_tensor("w_sz", [128, 128], F32, kind="ExternalInput")
    bias_d = nc.dram_tensor("biasv", [128, 1], F32, kind="ExternalInput")
    sinb_d = nc.dram_tensor("sinb", [64, 1], F32, kind="ExternalInput")
    y0s_d = nc.dram_tensor("y0s", [128, J], F32, kind="ExternalInput")
    sinit_d = nc.dram_tensor("sinit", [64, 2 * J], F32, kind="ExternalInput")
    zin_d = nc.dram_tensor("zin", [64, nt, J], F32, kind="ExternalInput")
    ophi_d = nc.dram_tensor("out_phi", [64, nslot, J], F32, kind="ExternalOutput")
    ov_d = nc.dram_tensor("out_v", [64, nslot, J], F32, kind="ExternalOutput")

    import contextlib

    ctx = contextlib.ExitStack()
    with ctx:
        w_a = ctx.enter_context(nc.sbuf_tensor("w_a_sb", [128, 128], F32))
        w_sz = ctx.enter_context(nc.sbuf_tensor("w_sz_sb", [128, 128], F32))
        biasv = ctx.enter_context(nc.sbuf_tensor("bias_sb", [128, 1], F32))
        sinb = ctx.enter_context(nc.sbuf_tensor("sinb_sb", [64, 1], F32))
        ybuf = [
            ctx.enter_context(nc.sbuf_tensor(f"ybuf{i}", [128, BS * J], F32))
            for i in range(3)
        ]
        x2buf = [
            ctx.enter_context(nc.sbuf_tensor(f"x2buf{i}", [128, BS * J], F32))
            for i in range(2)
        ]
        qbuf = ctx.enter_context(nc.sbuf_tensor("qbuf", [64, RR * J], F32))
        pbuf = ctx.enter_context(nc.sbuf_tensor("pbuf", [64, RR * J], F32))
        nbuf = ctx.enter_context(nc.sbuf_tensor("nbuf", [64, RR * J], F32))
        wbuf = ctx.enter_context(nc.sbuf_tensor("wbuf", [64, RR * J], F32))
        pb = [
            ctx.enter_context(nc.psum_tensor(f"pb{i}", [128, J], F32))
            for i in range(NBANK)
        ]
        s_upl = ctx.enter_context(nc.semaphore("s_upl"))
        s_zin = ctx.enter_context(nc.semaphore("s_zin"))
        s_pe = ctx.enter_context(nc.semaphore("s_pe"))
        s_dve = ctx.enter_context(nc.semaphore("s_dve"))
        s_q = ctx.enter_context(nc.semaphore("s_q"))
        s_pool = ctx.enter_context(nc.semaphore("s_pool"))
        s_act = ctx.enter_context(nc.semaphore("s_act"))
        s_yout = ctx.enter_context(nc.semaphore("s_yout"))
        block = ctx.enter_context(nc.Block())

        n_upl = 6  # w_a, w_sz, biasv, sinb, y0s, sinit

        @block.sync
        def _(sync):
            sync.dma_start(w_a[:, :], w_a_d[:, :]).then_inc(s_upl, 16)
            sync.dma_start(w_sz[:, :], w_sz_d[:, :]).then_inc(s_upl, 16)
            sync.dma_start(biasv[:, :], bias_d[:, :]).then_inc(s_upl, 16)
            sync.dma_start(sinb[:, :], sinb_d[:, :]).then_inc(s_upl, 16)
            sync.dma_start(ybuf[0][:, 0:J], y0s_d[:, :]).then_inc(s_upl, 16)
            sync.dma_start(x2buf[0][0:64, 0 : 2 * J], sinit_d[:, :]).then_inc(
                s_upl, 16
            )
            t1 = min(nt, BS)
            sync.dma_start(
                x2buf[0][64:128, 0 : t1 * J], zin_d[:, 0:t1, :]
            ).then_inc(s_zin, 16)

            for m in range(nblk):
                zm = m + 1
                if zm * BS < nt:
                    t0, t1 = zm * BS, min(nt, zm * BS + BS)
                    if m >= 1:
                        sync.wait_ge(s_pe, m * BS)
                    sync.dma_start(
                        x2buf[zm % 2][64:128, 0 : (t1 - t0) * J],
                        zin_d[:, t0:t1, :],
                    ).then_inc(s_zin, 16)
                if m >= 1:
                    b = m - 1
                    lastslot = min(nslot - 1, b * BS + BS - 1)
                    sync.wait_ge(s_dve, lastslot)
                    pn = min(BS, nslot - 1 - b * BS)
                    if pn > 0:
                        sync.dma_start(
                            ophi_d[:, b * BS + 1 : b * BS + 1 + pn, :],
                            ybuf[b % 3][0:64, 0 : pn * J].rearrange(
                                "p (t j) -> p t j", j=J
                            ),
                        ).then_inc(s_yout, 16)
                    vn = min(BS, nslot - b * BS)
                    sync.dma_start(
                        ov_d[:, b * BS : b * BS + vn, :],
                        ybuf[b % 3][64:128, 0 : vn * J].rearrange(
                            "p (t j) -> p t j", j=J
                        ),
                    ).then_inc(s_yout, 16)
            b = nblk - 1
            sync.wait_ge(s_dve, nt)
            pn = min(BS, nslot - 1 - b * BS)
            n_out = 2 * (nblk - 1) + 1
            if pn > 0:
                n_out += 1
                sync.dma_start(
                    ophi_d[:, b * BS + 1 : b * BS + 1 + pn, :],
                    ybuf[b % 3][0:64, 0 : pn * J].rearrange("p (t j) -> p t j", j=J),
                ).then_inc(s_yout, 16)
            vn = min(BS, nslot - b * BS)
            sync.dma_start(
                ov_d[:, b * BS : b * BS + vn, :],
                ybuf[b % 3][64:128, 0 : vn * J].rearrange("p (t j) -> p t j", j=J),
            ).then_inc(s_yout, 16)
            sync.wait_ge(s_yout, 16 * n_out)

        @block.tensor
        def _(pe):
            pe.wait_ge(s_upl, 16 * n_upl)
            for t in range(nt):
                m = t // BS
                c = (t % BS) * J
                if t % BS == 0:
                    pe.wait_ge(s_zin, 16 * (m + 1))
                if t >= 2:
                    pe.wait_ge(s_act, t - 1)
                pe.matmul(
                    pb[t % NBANK][:, :],
                    w_sz[:, :],
                    x2buf[m % 2][:, c : c + J],
                    start=True,
                    stop=False,
                )
                if t >= 1:
                    pe.wait_ge(s_dve, t)
                pe.matmul(
                    pb[t % NBANK][:, :],
                    w_a[:, :],
                    ybuf[m % 3][:, c : c + J],
                    start=False,
                    stop=True,
                ).then_inc(s_pe, 1)

        @block.vector
        def _(dve):
            for t in range(nt):
                s = t + 1
                mb = s // BS
                c2 = (s % BS) * J
                if s % BS == 0 and mb >= 3:
                    dve.wait_ge(s_yout, 32 * (mb - 2))
                dve.wait_ge(s_pe, t + 1)
                dve.tensor_scalar(
                    ybuf[mb % 3][:, c2 : c2 + J],
                    pb[t % NBANK][:, :],
                    biasv[:, 0:1],
                    None,
                    op0=A.add,
                ).then_inc(s_dve, 1)
                if t < nred:
                    # q_s = phi_{s+1}*ic + (2^23+256), slot s = t+1
                    rc = (s % RR) * J
                    if t >= RR - 1:
                        dve.wait_ge(s_pool, t - (RR - 2))  # qbuf slot reuse
                    dve.tensor_scalar(
                        qbuf[:, rc : rc + J],
                        ybuf[mb % 3][0:64, c2 : c2 + J],
                        IC,
                        B1,
                        op0=A.mult,
                        op1=A.add,
                    ).then_inc(s_q, 1)

        @block.gpsimd
        def _(pool):
            for t in range(nred):
                s = t + 1
                mb = s // BS
                c2 = (s % BS) * J
                rc = (s % RR) * J
                pool.wait_ge(s_q, t + 1)
                if t >= RR - 1:
                    pool.wait_ge(s_act, t - (RR - 2))  # wbuf slot reuse
                pool.tensor_scalar(
                    pbuf[:, rc : rc + J],
                    ybuf[mb % 3][0:64, c2 : c2 + J],
                    IC,
                    None,
                    op0=A.mult,
                )
                pool.tensor_scalar(
                    nbuf[:, rc : rc + J],
                    qbuf[:, rc : rc + J],
                    -B2,
                    None,
                    op0=A.add,
                )
                pool.tensor_sub(
                    wbuf[:, rc : rc + J],
                    pbuf[:, rc : rc + J],
                    nbuf[:, rc : rc + J],
                ).then_inc(s_pool, 1)

        @block.scalar
        def _(act):
            for t in range(nred):
                tgt = t + 2
                ma = tgt // BS
                ca = (tgt % BS) * J
                rc = ((t + 1) % RR) * J
                act.wait_ge(s_pool, t + 1)
                act.activation(
                    x2buf[ma % 2][0:64, ca : ca + J],
                    wbuf[:, rc : rc + J],
                    mybir.ActivationFunctionType.Sin,
                    bias=sinb[:, 0:1],
                    scale=C2PI,
                ).then_inc(s_act, 1)

    return nc


def _host_prep(params, y0, noise, T, N):
    f32 = np.float32
    nt = noise.shape[0]
    dt = f32(np.float32(T) / np.float32(int(N) - 1))
    d = float(dt)
    sqdt = float(np.sqrt(dt).astype(f32))
    p = np.asarray(params, dtype=np.float64)
    b1, b2, i1, i2, k1, k2, s1, s2 = (float(v) for v in p)

    cv1 = [-d * k1, d * k1, 1 - d * b1 + d * d * k1, -d * d * k1]
    cv2 = [d * k2, -d * k2, -d * d * k2, 1 - d * b2 + d * d * k2]
    cp1 = [1 + d * cv1[0], d * cv1[1], d * cv1[2], d * cv1[3]]
    cp2 = [d * cv2[0], 1 + d * cv2[1], d * cv2[2], d * cv2[3]]
    Am = np.array([cp1, cp2, cv1, cv2], dtype=f32)  # [out, in]
    SZ = np.array(
        [
            [-d * d, 0.0, -d, 0.0],
            [0.0, -d * d, 0.0, -d],
            [d * s1 * sqdt, 0.0, s1 * sqdt, 0.0],
            [0.0, d * s2 * sqdt, 0.0, s2 * sqdt],
        ],
        dtype=f32,
    )  # [in, out]

    eye = np.eye(G, dtype=f32)
    w_a = np.zeros((128, 128), dtype=f32)
    w_sz = np.zeros((128, 128), dtype=f32)
    for cin in range(4):
        for cout in range(4):
            w_a[cin * G : (cin + 1) * G, cout * G : (cout + 1) * G] = (
                Am[cout, cin] * eye
            )
            w_sz[cin * G : (cin + 1) * G, cout * G : (cout + 1) * G] = (
                SZ[cin, cout] * eye
            )
    biasv = np.empty((128, 1), dtype=f32)
    biasv[0:32] = f32(d * d * i1)
    biasv[32:64] = f32(d * d * i2)
    biasv[64:96] = f32(d * i1)
    biasv[96:128] = f32(d * i2)
    sinb = np.full((64, 1), np.float32(256.0 * C2PI), dtype=f32)

    y0 = np.asarray(y0, dtype=f32)
    noise = np.asarray(noise, dtype=f32)
    dtf = f32(dt)

    in_maps = []
    for ci in range(N_CORES):
        yc = y0[ci * BPC : (ci + 1) * BPC].reshape(G, J, 4)
        phi10, v10 = yc[:, :, 0], yc[:, :, 1]
        phi20, v20 = yc[:, :, 2], yc[:, :, 3]
        P1 = (phi10 + v10 * dtf).astype(f32)
        P2 = (phi20 + v20 * dtf).astype(f32)
        y0s = np.concatenate([P1, P2, v10, v20], axis=0)
        sinit = np.empty((64, 2 * J), dtype=f32)
        sinit[0:32, 0:J] = np.sin(phi10, dtype=f32)
        sinit[32:64, 0:J] = np.sin(phi20, dtype=f32)
        sinit[0:32, J : 2 * J] = np.sin(P1, dtype=f32)
        sinit[32:64, J : 2 * J] = np.sin(P2, dtype=f32)
        zc_block = noise[:, ci * BPC : (ci + 1) * BPC, :].reshape(nt, G, J, 2)
        zin = np.ascontiguousarray(zc_block.transpose(3, 1, 0, 2)).reshape(64, nt, J)
        in_maps.append(
            {
                "w_a": np.ascontiguousarray(w_a),
                "w_sz": np.ascontiguousarray(w_sz),
                "biasv": biasv,
                "sinb": sinb,
                "y0s": np.ascontiguousarray(y0s),
                "sinit": sinit,
                "zin": zin,
            }
        )
    return in_maps


def run_device(params, y0, noise, T, N, trace=False):
    nt = noise.shape[0]
    key = nt
    if key not in _CACHE:
        _CACHE[key] = _build_program(nt)
    nc = _CACHE[key]
    in_maps = _host_prep(params, y0, noise, T, N)
    res = bass_utils.run_bass_kernel_spmd(
        nc, in_maps, core_ids=list(range(N_CORES)), trace=trace
    )
    nslot = nt + 1
    traj = np.empty((BATCH, nslot, 4), dtype=np.float32)
    for ci in range(N_CORES):
        phi = res.results[ci]["out_phi"].reshape(2, G, nslot, J)
        v = res.results[ci]["out_v"].reshape(2, G, nslot, J)
        sl = slice(ci * BPC, (ci + 1) * BPC)
        traj[sl, :, 0] = phi[0].transpose(0, 2, 1).reshape(BPC, nslot)
        traj[sl, :, 2] = phi[1].transpose(0, 2, 1).reshape(BPC, nslot)
        traj[sl, :, 1] = v[0].transpose(0, 2, 1).reshape(BPC, nslot)
        traj[sl, :, 3] = v[1].transpose(0, 2, 1).reshape(BPC, nslot)
    traj[:, 0, :] = np.asarray(y0, dtype=np.float32)
    return res, traj


def kernel(params, y0, noise, T, N):
    res, traj = run_device(params, y0, noise, int(T), int(N))
    return traj


# revision 11
# speedup vs baseline: 1.1771x; 1.1771x over previous
"""Josephson-junction SDE Euler-Maruyama kernel for 8 Trainium2 NeuronCores.

Per core (batch 2048 = 32 groups x 64 columns), skewed state
Y~_t = [phi1_{t+1}, phi2_{t+1}, v1_t, v2_t] laid out one 32-partition block per
component. Substituting phi_t = phi_{t+1} - dt*v_t makes the update linear in
the skewed state, so each step is two fp32 matmuls into one PSUM bank
(MM2: sin/noise terms via W_SZ; MM1: drift via W_A) plus a DVE
tensor_scalar_add eviction that adds the per-partition dt*i constants.

sin(phi) needs |arg| <~ 3.3 for the ACT LUT while phi reaches ~600, so the
sine input is range-reduced on GPSIMD: q = phi/2pi + (2^23+256) rounds the
turn count via the magic-number trick, m = (q - (2^23+256)) * 2pi recovers
the wrap multiple exactly (Sterbenz subtract, then one rounding), and
w' = phi - m_stale is the reduced angle, using the PREVIOUS step's m (the
LUT stays accurate past pi by more than one step's phase drift, which keeps
the q/m pipeline off the sin critical chain). ACT computes S = Sin(w').
The skewed state gives the evict -> w' -> sin chain two steps of slack.

Everything is fully unrolled with static semaphore thresholds; noise streams
in and the trajectory streams out as ~1 MiB per-64-step block DMAs with
per-partition-contiguous DRAM layouts ([comp, g, t, j]).
"""

import math

import numpy as np

import concourse.bass as bass
import concourse.mybir as mybir
from concourse import bass_utils

F32 = mybir.dt.float32
A = mybir.AluOpType
N_CORES = 8
BATCH = 16384
BPC = BATCH // N_CORES  # 2048
G = 32  # partition groups per component
J = 64  # batch columns per step-slot
BS = 64  # steps per ring block
NBANK = 8
RR = 16  # reduction ring slots

IC = float(np.float32(1.0 / (2 * math.pi)))
C2PI = float(np.float32(2 * math.pi))
B1 = 8388864.0  # 2^23 + 256
B2 = 8388608.0  # 2^23

_CACHE = {}


def _build_program(nt):
    """Per-core bass program integrating nt steps (slots 0..nt)."""
    nslot = nt + 1
    nblk = (nslot + BS - 1) // BS
    nred = max(0, nt - 2)  # reduction chains: t = 0..nt-3 (slot t+1, S_{t+2})
    nc = bass.Bass()

    w_a_d = nc.dram_tensor("w_a", [128, 128], F32, kind="ExternalInput")
    w_sz_d = nc.dram_tensor("w_sz", [128, 128], F32, kind="ExternalInput")
    bias_d = nc.dram_tensor("biasv", [128, 1], F32, kind="ExternalInput")
    sinb_d = nc.dram_tensor("sinb", [64, 1], F32, kind="ExternalInput")
    y0s_d = nc.dram_tensor("y0s", [128, J], F32, kind="ExternalInput")
    sinit_d = nc.dram_tensor("sinit", [64, 2 * J], F32, kind="ExternalInput")
    zin_d = nc.dram_tensor("zin", [64, nt, J], F32, kind="ExternalInput")
    ophi_d = nc.dram_tensor("out_phi", [64, nslot, J], F32, kind="ExternalOutput")
    ov_d = nc.dram_tensor("out_v", [64, nslot, J], F32, kind="ExternalOutput")

    import contextlib

    ctx = contextlib.ExitStack()
    with ctx:
        w_a = ctx.enter_context(nc.sbuf_tensor("w_a_sb", [128, 128], F32))
        w_sz = ctx.enter_context(nc.sbuf_tensor("w_sz_sb", [128, 128], F32))
        biasv = ctx.enter_context(nc.sbuf_tensor("bias_sb", [128, 1], F32))
        sinb = ctx.enter_context(nc.sbuf_tensor("sinb_sb", [64, 1], F32))
        ybuf = [
            ctx.enter_context(nc.sbuf_tensor(f"ybuf{i}", [128, BS * J], F32))
            for i in range(3)
        ]
        x2buf = [
            ctx.enter_context(nc.sbuf_tensor(f"x2buf{i}", [128, BS * J], F32))
            for i in range(2)
        ]
        pbuf = ctx.enter_context(nc.sbuf_tensor("pbuf", [64, RR * J], F32))
        nbuf = ctx.enter_context(nc.sbuf_tensor("nbuf", [64, RR * J], F32))
        wbuf = ctx.enter_context(nc.sbuf_tensor("wbuf", [64, RR * J], F32))
        pb = [
            ctx.enter_context(nc.psum_tensor(f"pb{i}", [128, J], F32))
            for i in range(NBANK)
        ]
        s_upl = ctx.enter_context(nc.semaphore("s_upl"))
        s_zin = ctx.enter_context(nc.semaphore("s_zin"))
        s_pe = ctx.enter_context(nc.semaphore("s_pe"))
        s_dve = ctx.enter_context(nc.semaphore("s_dve"))
        s_pool = ctx.enter_context(nc.semaphore("s_pool"))
        s_act = ctx.enter_context(nc.semaphore("s_act"))
        s_yout = ctx.enter_context(nc.semaphore("s_yout"))
        block = ctx.enter_context(nc.Block())

        n_upl = 6  # w_a, w_sz, biasv, sinb, y0s, sinit

        @block.sync
        def _(sync):
            sync.dma_start(w_a[:, :], w_a_d[:, :]).then_inc(s_upl, 16)
            sync.dma_start(w_sz[:, :], w_sz_d[:, :]).then_inc(s_upl, 16)
            sync.dma_start(biasv[:, :], bias_d[:, :]).then_inc(s_upl, 16)
            sync.dma_start(sinb[:, :], sinb_d[:, :]).then_inc(s_upl, 16)
            sync.dma_start(ybuf[0][:, 0:J], y0s_d[:, :]).then_inc(s_upl, 16)
            sync.dma_start(x2buf[0][0:64, 0 : 2 * J], sinit_d[:, :]).then_inc(
                s_upl, 16
            )
            t1 = min(nt, BS)
            sync.dma_start(
                x2buf[0][64:128, 0 : t1 * J], zin_d[:, 0:t1, :]
            ).then_inc(s_zin, 16)

            for m in range(nblk):
                zm = m + 1
                if zm * BS < nt:
                    t0, t1 = zm * BS, min(nt, zm * BS + BS)
                    if m >= 1:
                        sync.wait_ge(s_pe, m * BS)
                    sync.dma_start(
                        x2buf[zm % 2][64:128, 0 : (t1 - t0) * J],
                        zin_d[:, t0:t1, :],
                    ).then_inc(s_zin, 16)
                if m >= 1:
                    b = m - 1
                    lastslot = min(nslot - 1, b * BS + BS - 1)
                    sync.wait_ge(s_dve, lastslot)
                    pn = min(BS, nslot - 1 - b * BS)
                    if pn > 0:
                        sync.dma_start(
                            ophi_d[:, b * BS + 1 : b * BS + 1 + pn, :],
                            ybuf[b % 3][0:64, 0 : pn * J].rearrange(
                                "p (t j) -> p t j", j=J
                            ),
                        ).then_inc(s_yout, 16)
                    vn = min(BS, nslot - b * BS)
                    sync.dma_start(
                        ov_d[:, b * BS : b * BS + vn, :],
                        ybuf[b % 3][64:128, 0 : vn * J].rearrange(
                            "p (t j) -> p t j", j=J
                        ),
                    ).then_inc(s_yout, 16)
            b = nblk - 1
            sync.wait_ge(s_dve, nt)
            pn = min(BS, nslot - 1 - b * BS)
            n_out = 2 * (nblk - 1) + 1
            if pn > 0:
                n_out += 1
                sync.dma_start(
                    ophi_d[:, b * BS + 1 : b * BS + 1 + pn, :],
                    ybuf[b % 3][0:64, 0 : pn * J].rearrange("p (t j) -> p t j", j=J),
                ).then_inc(s_yout, 16)
            vn = min(BS, nslot - b * BS)
            sync.dma_start(
                ov_d[:, b * BS : b * BS + vn, :],
                ybuf[b % 3][64:128, 0 : vn * J].rearrange("p (t j) -> p t j", j=J),
            ).then_inc(s_yout, 16)
            sync.wait_ge(s_yout, 16 * n_out)

        @block.tensor
        def _(pe):
            pe.wait_ge(s_upl, 16 * n_upl)
            for t in range(nt):
                m = t // BS
                c = (t % BS) * J
                if t % BS == 0:
                    pe.wait_ge(s_zin, 16 * (m + 1))
                if t >= 2:
                    pe.wait_ge(s_act, t - 1)
                pe.matmul(
                    pb[t % NBANK][:, :],
                    w_sz[:, :],
                    x2buf[m % 2][:, c : c + J],
                    start=True,
                    stop=False,
                )
                if t >= 1:
                    pe.wait_ge(s_dve, t)
                pe.matmul(
                    pb[t % NBANK][:, :],
                    w_a[:, :],
                    ybuf[m % 3][:, c : c + J],
                    start=False,
                    stop=True,
                ).then_inc(s_pe, 1)

        @block.vector
        def _(dve):
            for t in range(nt):
                s = t + 1
                mb = s // BS
                c2 = (s % BS) * J
                if s % BS == 0 and mb >= 3:
                    dve.wait_ge(s_yout, 32 * (mb - 2))
                dve.wait_ge(s_pe, t + 1)
                dve.tensor_scalar(
                    ybuf[mb % 3][:, c2 : c2 + J],
                    pb[t % NBANK][:, :],
                    biasv[:, 0:1],
                    None,
                    op0=A.add,
                ).then_inc(s_dve, 1)

        @block.gpsimd
        def _(pool):
            # w'_t = phi_{t+2} - m_{t-1} in radians, with the one-step-stale
            # wrap multiple m = round(phi/2pi + const)*2pi: the ACT sin LUT is
            # accurate past pi by more than one step's phase drift, so the
            # stale wrap keeps the q -> m computation off the sin chain.
            # pbuf holds q, nbuf holds m.
            for t in range(nred):
                s = t + 1
                mb = s // BS
                c2 = (s % BS) * J
                rc = (s % RR) * J
                rp = ((s - 1) % RR) * J if t >= 1 else rc
                pool.wait_ge(s_dve, t + 1)
                if t >= RR - 1:
                    pool.wait_ge(s_act, t - (RR - 2))  # wbuf slot reuse
                if t == 0:
                    pool.tensor_scalar(
                        pbuf[:, rc : rc + J],
                        ybuf[mb % 3][0:64, c2 : c2 + J],
                        IC, B1, op0=A.mult, op1=A.add,
                    )
                    pool.tensor_scalar(
                        nbuf[:, rc : rc + J], pbuf[:, rc : rc + J],
                        -B1, C2PI, op0=A.add, op1=A.mult,
                    )
                pool.tensor_sub(
                    wbuf[:, rc : rc + J],
                    ybuf[mb % 3][0:64, c2 : c2 + J],
                    nbuf[:, rp : rp + J],
                ).then_inc(s_pool, 1)
                # off-critical: q_t and m_t for the next step's w'
                pool.tensor_scalar(
                    pbuf[:, rc : rc + J],
                    ybuf[mb % 3][0:64, c2 : c2 + J],
                    IC, B1, op0=A.mult, op1=A.add,
                )
                pool.tensor_scalar(
                    nbuf[:, rc : rc + J], pbuf[:, rc : rc + J],
                    -B1, C2PI, op0=A.add, op1=A.mult,
                )

        @block.scalar
        def _(act):
            for t in range(nred):
                tgt = t + 2
                ma = tgt // BS
                ca = (tgt % BS) * J
                rc = ((t + 1) % RR) * J
                act.wait_ge(s_pool, t + 1)
                act.activation(
                    x2buf[ma % 2][0:64, ca : ca + J],
                    wbuf[:, rc : rc + J],
                    mybir.ActivationFunctionType.Sin,
                    bias=sinb[:, 0:1],
                    scale=1.0,
                ).then_inc(s_act, 1)

    return nc


def _host_prep(params, y0, noise, T, N):
    f32 = np.float32
    nt = noise.shape[0]
    dt = f32(np.float32(T) / np.float32(int(N) - 1))
    d = float(dt)
    sqdt = float(np.sqrt(dt).astype(f32))
    p = np.asarray(params, dtype=np.float64)
    b1, b2, i1, i2, k1, k2, s1, s2 = (float(v) for v in p)

    cv1 = [-d * k1, d * k1, 1 - d * b1 + d * d * k1, -d * d * k1]
    cv2 = [d * k2, -d * k2, -d * d * k2, 1 - d * b2 + d * d * k2]
    cp1 = [1 + d * cv1[0], d * cv1[1], d * cv1[2], d * cv1[3]]
    cp2 = [d * cv2[0], 1 + d * cv2[1], d * cv2[2], d * cv2[3]]
    Am = np.array([cp1, cp2, cv1, cv2], dtype=f32)  # [out, in]
    SZ = np.array(
        [
            [-d * d, 0.0, -d, 0.0],
            [0.0, -d * d, 0.0, -d],
            [d * s1 * sqdt, 0.0, s1 * sqdt, 0.0],
            [0.0, d * s2 * sqdt, 0.0, s2 * sqdt],
        ],
        dtype=f32,
    )  # [in, out]

    eye = np.eye(G, dtype=f32)
    w_a = np.zeros((128, 128), dtype=f32)
    w_sz = np.zeros((128, 128), dtype=f32)
    for cin in range(4):
        for cout in range(4):
            w_a[cin * G : (cin + 1) * G, cout * G : (cout + 1) * G] = (
                Am[cout, cin] * eye
            )
            w_sz[cin * G : (cin + 1) * G, cout * G : (cout + 1) * G] = (
                SZ[cin, cout] * eye
            )
    biasv = np.empty((128, 1), dtype=f32)
    biasv[0:32] = f32(d * d * i1)
    biasv[32:64] = f32(d * d * i2)
    biasv[64:96] = f32(d * i1)
    biasv[96:128] = f32(d * i2)
    sinb = np.zeros((64, 1), dtype=f32)  # sin bias (w' is already in radians)

    y0 = np.asarray(y0, dtype=f32)
    noise = np.asarray(noise, dtype=f32)
    dtf = f32(dt)

    in_maps = []
    for ci in range(N_CORES):
        yc = y0[ci * BPC : (ci + 1) * BPC].reshape(G, J, 4)
        phi10, v10 = yc[:, :, 0], yc[:, :, 1]
        phi20, v20 = yc[:, :, 2], yc[:, :, 3]
        P1 = (phi10 + v10 * dtf).astype(f32)
        P2 = (phi20 + v20 * dtf).astype(f32)
        y0s = np.concatenate([P1, P2, v10, v20], axis=0)
        sinit = np.empty((64, 2 * J), dtype=f32)
        sinit[0:32, 0:J] = np.sin(phi10, dtype=f32)
        sinit[32:64, 0:J] = np.sin(phi20, dtype=f32)
        sinit[0:32, J : 2 * J] = np.sin(P1, dtype=f32)
        sinit[32:64, J : 2 * J] = np.sin(P2, dtype=f32)
        zc_block = noise[:, ci * BPC : (ci + 1) * BPC, :].reshape(nt, G, J, 2)
        zin = np.ascontiguousarray(zc_block.transpose(3, 1, 0, 2)).reshape(64, nt, J)
        in_maps.append(
            {
                "w_a": np.ascontiguousarray(w_a),
                "w_sz": np.ascontiguousarray(w_sz),
                "biasv": biasv,
                "sinb": sinb,
                "y0s": np.ascontiguousarray(y0s),
                "sinit": sinit,
                "zin": zin,
            }
        )
    return in_maps


def run_device(params, y0, noise, T, N, trace=False):
    nt = noise.shape[0]
    key = nt
    if key not in _CACHE:
        _CACHE[key] = _build_program(nt)
    nc = _CACHE[key]
    in_maps = _host_prep(params, y0, noise, T, N)
    res = bass_utils.run_bass_kernel_spmd(
        nc, in_maps, core_ids=list(range(N_CORES)), trace=trace
    )
    nslot = nt + 1
    traj = np.empty((BATCH, nslot, 4), dtype=np.float32)
    for ci in range(N_CORES):
        phi = res.results[ci]["out_phi"].reshape(2, G, nslot, J)
        v = res.results[ci]["out_v"].reshape(2, G, nslot, J)
        sl = slice(ci * BPC, (ci + 1) * BPC)
        traj[sl, :, 0] = phi[0].transpose(0, 2, 1).reshape(BPC, nslot)
        traj[sl, :, 2] = phi[1].transpose(0, 2, 1).reshape(BPC, nslot)
        traj[sl, :, 1] = v[0].transpose(0, 2, 1).reshape(BPC, nslot)
        traj[sl, :, 3] = v[1].transpose(0, 2, 1).reshape(BPC, nslot)
    traj[:, 0, :] = np.asarray(y0, dtype=np.float32)
    return res, traj


def kernel(params, y0, noise, T, N):
    res, traj = run_device(params, y0, noise, int(T), int(N))
    return traj


# revision 15
# speedup vs baseline: 1.2048x; 1.0235x over previous
"""Josephson-junction SDE Euler-Maruyama kernel for 8 Trainium2 NeuronCores.

Per core (batch 2048 = 32 groups x 64 columns), skewed state
Y~_t = [phi1_{t+1}, phi2_{t+1}, v1_t, v2_t] laid out one 32-partition block per
component. Substituting phi_t = phi_{t+1} - dt*v_t makes the update linear in
the skewed state, so each step is two fp32 matmuls into one PSUM bank
(MM2: sin/noise terms via W_SZ; MM1: drift via W_A) plus a DVE
tensor_scalar_add eviction that adds the per-partition dt*i constants.

sin(phi) needs |arg| <~ 3.3 for the ACT LUT while phi reaches ~600, so the
sine input is range-reduced on GPSIMD: q = phi/2pi + (2^23+256) rounds the
turn count via the magic-number trick, m = (q - (2^23+256)) * 2pi recovers
the wrap multiple exactly (Sterbenz subtract, then one rounding), and
w' = phi - m_stale is the reduced angle, using the PREVIOUS step's m (the
LUT stays accurate past pi by more than one step's phase drift, which keeps
the q/m pipeline off the sin critical chain). ACT computes S = Sin(w').
The skewed state gives the evict -> w' -> sin chain two steps of slack.

Everything is fully unrolled with static semaphore thresholds; noise streams
in and the trajectory streams out as ~1 MiB per-64-step block DMAs with
per-partition-contiguous DRAM layouts ([comp, g, t, j]).
"""

import math

import numpy as np

import concourse.bass as bass
import concourse.mybir as mybir
from concourse import bass_utils

F32 = mybir.dt.float32
A = mybir.AluOpType
N_CORES = 8
BATCH = 16384
BPC = BATCH // N_CORES  # 2048
G = 32  # partition groups per component
J = 64  # batch columns per step-slot
BS = 64  # steps per ring block
NBANK = 8
RR = 16  # reduction ring slots

IC = float(np.float32(1.0 / (2 * math.pi)))
C2PI = float(np.float32(2 * math.pi))
B1 = 8388864.0  # 2^23 + 256
B2 = 8388608.0  # 2^23

_CACHE = {}


def _build_program(nt):
    """Per-core bass program integrating nt steps (slots 0..nt)."""
    nslot = nt + 1
    nblk = (nslot + BS - 1) // BS
    nred = max(0, nt - 2)  # reduction chains: t = 0..nt-3 (slot t+1, S_{t+2})
    nc = bass.Bass()

    w_a_d = nc.dram_tensor("w_a", [128, 128], F32, kind="ExternalInput")
    w_sz_d = nc.dram_tensor("w_sz", [128, 128], F32, kind="ExternalInput")
    bias_d = nc.dram_tensor("biasv", [128, 1], F32, kind="ExternalInput")
    sinb_d = nc.dram_tensor("sinb", [64, 1], F32, kind="ExternalInput")
    y0s_d = nc.dram_tensor("y0s", [128, J], F32, kind="ExternalInput")
    sinit_d = nc.dram_tensor("sinit", [64, 2 * J], F32, kind="ExternalInput")
    zin_d = nc.dram_tensor("zin", [64, nt, J], F32, kind="ExternalInput")
    ophi_d = nc.dram_tensor("out_phi", [64, nslot, J], F32, kind="ExternalOutput")
    ov_d = nc.dram_tensor("out_v", [64, nslot, J], F32, kind="ExternalOutput")

    import contextlib

    ctx = contextlib.ExitStack()
    with ctx:
        w_a = ctx.enter_context(nc.sbuf_tensor("w_a_sb", [128, 128], F32))
        w_sz = ctx.enter_context(nc.sbuf_tensor("w_sz_sb", [128, 128], F32))
        biasv = ctx.enter_context(nc.sbuf_tensor("bias_sb", [128, 1], F32))
        sinb = ctx.enter_context(nc.sbuf_tensor("sinb_sb", [64, 1], F32))
        ybuf = [
            ctx.enter_context(nc.sbuf_tensor(f"ybuf{i}", [128, BS * J], F32))
            for i in range(3)
        ]
        x2buf = [
            ctx.enter_context(nc.sbuf_tensor(f"x2buf{i}", [128, BS * J], F32))
            for i in range(2)
        ]
        pbuf = ctx.enter_context(nc.sbuf_tensor("pbuf", [64, RR * J], F32))
        nbuf = ctx.enter_context(nc.sbuf_tensor("nbuf", [64, RR * J], F32))
        wbuf = ctx.enter_context(nc.sbuf_tensor("wbuf", [64, RR * J], F32))
        pb = [
            ctx.enter_context(nc.psum_tensor(f"pb{i}", [128, J], F32))
            for i in range(NBANK)
        ]
        s_upl = ctx.enter_context(nc.semaphore("s_upl"))
        s_zin = ctx.enter_context(nc.semaphore("s_zin"))
        s_pe = ctx.enter_context(nc.semaphore("s_pe"))
        s_dve = ctx.enter_context(nc.semaphore("s_dve"))
        s_pool = ctx.enter_context(nc.semaphore("s_pool"))
        s_m = ctx.enter_context(nc.semaphore("s_m"))
        s_act = ctx.enter_context(nc.semaphore("s_act"))
        s_yout = ctx.enter_context(nc.semaphore("s_yout"))
        block = ctx.enter_context(nc.Block())

        n_upl = 6  # w_a, w_sz, biasv, sinb, y0s, sinit

        @block.sync
        def _(sync):
            sync.dma_start(w_a[:, :], w_a_d[:, :]).then_inc(s_upl, 16)
            sync.dma_start(w_sz[:, :], w_sz_d[:, :]).then_inc(s_upl, 16)
            sync.dma_start(biasv[:, :], bias_d[:, :]).then_inc(s_upl, 16)
            sync.dma_start(sinb[:, :], sinb_d[:, :]).then_inc(s_upl, 16)
            sync.dma_start(ybuf[0][:, 0:J], y0s_d[:, :]).then_inc(s_upl, 16)
            sync.dma_start(x2buf[0][0:64, 0 : 2 * J], sinit_d[:, :]).then_inc(
                s_upl, 16
            )
            t1 = min(nt, BS)
            sync.dma_start(
                x2buf[0][64:128, 0 : t1 * J], zin_d[:, 0:t1, :]
            ).then_inc(s_zin, 16)

            for m in range(nblk):
                zm = m + 1
                if zm * BS < nt:
                    t0, t1 = zm * BS, min(nt, zm * BS + BS)
                    if m >= 1:
                        sync.wait_ge(s_pe, m * BS)
                    sync.dma_start(
                        x2buf[zm % 2][64:128, 0 : (t1 - t0) * J],
                        zin_d[:, t0:t1, :],
                    ).then_inc(s_zin, 16)
                if m >= 1:
                    b = m - 1
                    lastslot = min(nslot - 1, b * BS + BS - 1)
                    sync.wait_ge(s_dve, lastslot)
                    pn = min(BS, nslot - 1 - b * BS)
                    if pn > 0:
                        sync.dma_start(
                            ophi_d[:, b * BS + 1 : b * BS + 1 + pn, :],
                            ybuf[b % 3][0:64, 0 : pn * J].rearrange(
                                "p (t j) -> p t j", j=J
                            ),
                        ).then_inc(s_yout, 16)
                    vn = min(BS, nslot - b * BS)
                    sync.dma_start(
                        ov_d[:, b * BS : b * BS + vn, :],
                        ybuf[b % 3][64:128, 0 : vn * J].rearrange(
                            "p (t j) -> p t j", j=J
                        ),
                    ).then_inc(s_yout, 16)
            b = nblk - 1
            sync.wait_ge(s_dve, nt)
            pn = min(BS, nslot - 1 - b * BS)
            n_out = 2 * (nblk - 1) + 1
            if pn > 0:
                n_out += 1
                sync.dma_start(
                    ophi_d[:, b * BS + 1 : b * BS + 1 + pn, :],
                    ybuf[b % 3][0:64, 0 : pn * J].rearrange("p (t j) -> p t j", j=J),
                ).then_inc(s_yout, 16)
            vn = min(BS, nslot - b * BS)
            sync.dma_start(
                ov_d[:, b * BS : b * BS + vn, :],
                ybuf[b % 3][64:128, 0 : vn * J].rearrange("p (t j) -> p t j", j=J),
            ).then_inc(s_yout, 16)
            sync.wait_ge(s_yout, 16 * n_out)

        @block.tensor
        def _(pe):
            pe.wait_ge(s_upl, 16 * n_upl)
            for t in range(nt):
                m = t // BS
                c = (t % BS) * J
                if t % BS == 0:
                    pe.wait_ge(s_zin, 16 * (m + 1))
                if t >= 2:
                    pe.wait_ge(s_act, t - 1)
                pe.matmul(
                    pb[t % NBANK][:, :],
                    w_sz[:, :],
                    x2buf[m % 2][:, c : c + J],
                    start=True,
                    stop=False,
                )
                if t >= 1:
                    pe.wait_ge(s_dve, t)
                pe.matmul(
                    pb[t % NBANK][:, :],
                    w_a[:, :],
                    ybuf[m % 3][:, c : c + J],
                    start=False,
                    stop=True,
                ).then_inc(s_pe, 1)

        @block.vector
        def _(dve):
            for t in range(nt):
                s = t + 1
                mb = s // BS
                c2 = (s % BS) * J
                if s % BS == 0 and mb >= 3:
                    dve.wait_ge(s_yout, 32 * (mb - 2))
                dve.wait_ge(s_pe, t + 1)
                dve.tensor_scalar(
                    ybuf[mb % 3][:, c2 : c2 + J],
                    pb[t % NBANK][:, :],
                    biasv[:, 0:1],
                    None,
                    op0=A.add,
                ).then_inc(s_dve, 1)
                if t < nred:
                    rc = (s % RR) * J
                    rp = ((s - 1) % RR) * J if t >= 1 else rc
                    dve.wait_ge(s_m, max(1, t))
                    if t >= RR - 1:
                        dve.wait_ge(s_act, t - (RR - 2))  # wbuf slot reuse
                    dve.scalar_tensor_tensor(
                        wbuf[:, rc : rc + J],
                        ybuf[mb % 3][0:64, c2 : c2 + J],
                        0.0,
                        nbuf[:, rp : rp + J],
                        op0=A.add,
                        op1=A.subtract,
                    ).then_inc(s_pool, 1)

        @block.gpsimd
        def _(pool):
            # off-critical wrap pipeline: q = phi/2pi + (2^23+256) rounds the
            # turn count; m = (q - (2^23+256)) * 2pi recovers the wrap multiple
            # (exact Sterbenz subtract, then one rounding). DVE consumes the
            # one-step-stale m in its w' = phi - m STT.
            for t in range(nred):
                s = t + 1
                mb = s // BS
                c2 = (s % BS) * J
                rc = (s % RR) * J
                pool.wait_ge(s_dve, t + 1)
                if t >= RR - 1:
                    pool.wait_ge(s_pool, t - (RR - 2))  # nbuf slot reuse
                pool.tensor_scalar(
                    pbuf[:, rc : rc + J],
                    ybuf[mb % 3][0:64, c2 : c2 + J],
                    IC, B1, op0=A.mult, op1=A.add,
                )
                pool.tensor_scalar(
                    nbuf[:, rc : rc + J], pbuf[:, rc : rc + J],
                    -B1, C2PI, op0=A.add, op1=A.mult,
                ).then_inc(s_m, 1)

        @block.scalar
        def _(act):
            for t in range(nred):
                tgt = t + 2
                ma = tgt // BS
                ca = (tgt % BS) * J
                rc = ((t + 1) % RR) * J
                act.wait_ge(s_pool, t + 1)
                act.activation(
                    x2buf[ma % 2][0:64, ca : ca + J],
                    wbuf[:, rc : rc + J],
                    mybir.ActivationFunctionType.Sin,
                    bias=sinb[:, 0:1],
                    scale=1.0,
                ).then_inc(s_act, 1)

    return nc


def _host_prep(params, y0, noise, T, N):
    f32 = np.float32
    nt = noise.shape[0]
    dt = f32(np.float32(T) / np.float32(int(N) - 1))
    d = float(dt)
    sqdt = float(np.sqrt(dt).astype(f32))
    p = np.asarray(params, dtype=np.float64)
    b1, b2, i1, i2, k1, k2, s1, s2 = (float(v) for v in p)

    cv1 = [-d * k1, d * k1, 1 - d * b1 + d * d * k1, -d * d * k1]
    cv2 = [d * k2, -d * k2, -d * d * k2, 1 - d * b2 + d * d * k2]
    cp1 = [1 + d * cv1[0], d * cv1[1], d * cv1[2], d * cv1[3]]
    cp2 = [d * cv2[0], 1 + d * cv2[1], d * cv2[2], d * cv2[3]]
    Am = np.array([cp1, cp2, cv1, cv2], dtype=f32)  # [out, in]
    SZ = np.array(
        [
            [-d * d, 0.0, -d, 0.0],
            [0.0, -d * d, 0.0, -d],
            [d * s1 * sqdt, 0.0, s1 * sqdt, 0.0],
            [0.0, d * s2 * sqdt, 0.0, s2 * sqdt],
        ],
        dtype=f32,
    )  # [in, out]

    eye = np.eye(G, dtype=f32)
    w_a = np.zeros((128, 128), dtype=f32)
    w_sz = np.zeros((128, 128), dtype=f32)
    for cin in range(4):
        for cout in range(4):
            w_a[cin * G : (cin + 1) * G, cout * G : (cout + 1) * G] = (
                Am[cout, cin] * eye
            )
            w_sz[cin * G : (cin + 1) * G, cout * G : (cout + 1) * G] = (
                SZ[cin, cout] * eye
            )
    biasv = np.empty((128, 1), dtype=f32)
    biasv[0:32] = f32(d * d * i1)
    biasv[32:64] = f32(d * d * i2)
    biasv[64:96] = f32(d * i1)
    biasv[96:128] = f32(d * i2)
    sinb = np.zeros((64, 1), dtype=f32)  # sin bias (w' is already in radians)

    y0 = np.asarray(y0, dtype=f32)
    noise = np.asarray(noise, dtype=f32)
    dtf = f32(dt)

    in_maps = []
    for ci in range(N_CORES):
        yc = y0[ci * BPC : (ci + 1) * BPC].reshape(G, J, 4)
        phi10, v10 = yc[:, :, 0], yc[:, :, 1]
        phi20, v20 = yc[:, :, 2], yc[:, :, 3]
        P1 = (phi10 + v10 * dtf).astype(f32)
        P2 = (phi20 + v20 * dtf).astype(f32)
        y0s = np.concatenate([P1, P2, v10, v20], axis=0)
        sinit = np.empty((64, 2 * J), dtype=f32)
        sinit[0:32, 0:J] = np.sin(phi10, dtype=f32)
        sinit[32:64, 0:J] = np.sin(phi20, dtype=f32)
        sinit[0:32, J : 2 * J] = np.sin(P1, dtype=f32)
        sinit[32:64, J : 2 * J] = np.sin(P2, dtype=f32)
        zc_block = noise[:, ci * BPC : (ci + 1) * BPC, :].reshape(nt, G, J, 2)
        zin = np.ascontiguousarray(zc_block.transpose(3, 1, 0, 2)).reshape(64, nt, J)
        in_maps.append(
            {
                "w_a": np.ascontiguousarray(w_a),
                "w_sz": np.ascontiguousarray(w_sz),
                "biasv": biasv,
                "sinb": sinb,
                "y0s": np.ascontiguousarray(y0s),
                "sinit": sinit,
                "zin": zin,
            }
        )
    return in_maps


def run_device(params, y0, noise, T, N, trace=False):
    nt = noise.shape[0]
    key = nt
    if key not in _CACHE:
        _CACHE[key] = _build_program(nt)
    nc = _CACHE[key]
    in_maps = _host_prep(params, y0, noise, T, N)
    res = bass_utils.run_bass_kernel_spmd(
        nc, in_maps, core_ids=list(range(N_CORES)), trace=trace
    )
    nslot = nt + 1
    traj = np.empty((BATCH, nslot, 4), dtype=np.float32)
    for ci in range(N_CORES):
        phi = res.results[ci]["out_phi"].reshape(2, G, nslot, J)
        v = res.results[ci]["out_v"].reshape(2, G, nslot, J)
        sl = slice(ci * BPC, (ci + 1) * BPC)
        traj[sl, :, 0] = phi[0].transpose(0, 2, 1).reshape(BPC, nslot)
        traj[sl, :, 2] = phi[1].transpose(0, 2, 1).reshape(BPC, nslot)
        traj[sl, :, 1] = v[0].transpose(0, 2, 1).reshape(BPC, nslot)
        traj[sl, :, 3] = v[1].transpose(0, 2, 1).reshape(BPC, nslot)
    traj[:, 0, :] = np.asarray(y0, dtype=np.float32)
    return res, traj


def kernel(params, y0, noise, T, N):
    res, traj = run_device(params, y0, noise, int(T), int(N))
    return traj


# revision 19
# speedup vs baseline: 1.3322x; 1.1058x over previous
"""Josephson-junction SDE Euler-Maruyama kernel for 8 Trainium2 NeuronCores.

Per core (batch 2048 = 32 groups x 64 columns), skewed state
Y~_t = [phi1_{t+1}, phi2_{t+1}, v1_t, v2_t] laid out one 32-partition block per
component. Substituting phi_t = phi_{t+1} - dt*v_t makes the update linear in
the skewed state. Steps t >= 2 use the COMPOSED two-step form
  Y~_{t+1} = A^2 * Y~_{t-1} + (A*SZ) * X2_{t-1} + SZ * X2_t + (A+I)b
as three fp32 matmuls into one PSUM bank, so the state matmul reads a slot
evicted TWO steps ago (halving the PE<->DVE round-trip pressure) and the
sin-gated matmul is last in the group. A DVE tensor_scalar eviction adds the
per-partition bias constants ((A+I)b for composed steps, b for steps 0-1,
which use the plain single-step weights).

sin(phi) needs |arg| <~ 3.3 for the ACT LUT while phi reaches ~600, so the
sine input is range-reduced on GPSIMD: q = phi/2pi + (2^23+256) rounds the
turn count via the magic-number trick, m = (q - (2^23+256)) * 2pi recovers
the wrap multiple exactly (Sterbenz subtract, then one rounding), and
w' = phi - m_stale is the reduced angle, using the PREVIOUS step's m (the
LUT stays accurate past pi by more than one step's phase drift, which keeps
the q/m pipeline off the sin critical chain). ACT computes S = Sin(w').
The skewed state gives the evict -> w' -> sin chain two steps of slack.

Everything is fully unrolled with static semaphore thresholds; noise streams
in and the trajectory streams out as ~1 MiB per-64-step block DMAs with
per-partition-contiguous DRAM layouts ([comp, g, t, j]).
"""

import math

import numpy as np

import concourse.bass as bass
import concourse.mybir as mybir
from concourse import bass_utils

F32 = mybir.dt.float32
A = mybir.AluOpType
N_CORES = 8
BATCH = 16384
BPC = BATCH // N_CORES  # 2048
G = 32  # partition groups per component
J = 64  # batch columns per step-slot
BS = 64  # steps per ring block
NBANK = 8
RR = 16  # reduction ring slots

IC = float(np.float32(1.0 / (2 * math.pi)))
C2PI = float(np.float32(2 * math.pi))
B1 = 8388864.0  # 2^23 + 256
B2 = 8388608.0  # 2^23

_CACHE = {}


def _build_program(nt):
    """Per-core bass program integrating nt steps (slots 0..nt)."""
    nslot = nt + 1
    nblk = (nslot + BS - 1) // BS
    nred = max(0, nt - 2)  # reduction chains: t = 0..nt-3 (slot t+1, S_{t+2})
    nc = bass.Bass()

    w_a_d = nc.dram_tensor("w_a", [128, 128], F32, kind="ExternalInput")
    w_sz_d = nc.dram_tensor("w_sz", [128, 128], F32, kind="ExternalInput")
    w_a2_d = nc.dram_tensor("w_a2", [128, 128], F32, kind="ExternalInput")
    w_sza_d = nc.dram_tensor("w_sza", [128, 128], F32, kind="ExternalInput")
    bias_d = nc.dram_tensor("biasv", [128, 1], F32, kind="ExternalInput")
    bias0_d = nc.dram_tensor("biasv0", [128, 1], F32, kind="ExternalInput")
    sinb_d = nc.dram_tensor("sinb", [64, 1], F32, kind="ExternalInput")
    y0s_d = nc.dram_tensor("y0s", [128, J], F32, kind="ExternalInput")
    sinit_d = nc.dram_tensor("sinit", [64, 2 * J], F32, kind="ExternalInput")
    zin_d = nc.dram_tensor("zin", [64, nt, J], F32, kind="ExternalInput")
    ophi_d = nc.dram_tensor("out_phi", [64, nslot, J], F32, kind="ExternalOutput")
    ov_d = nc.dram_tensor("out_v", [64, nslot, J], F32, kind="ExternalOutput")

    import contextlib

    ctx = contextlib.ExitStack()
    with ctx:
        w_a = ctx.enter_context(nc.sbuf_tensor("w_a_sb", [128, 128], F32))
        w_sz = ctx.enter_context(nc.sbuf_tensor("w_sz_sb", [128, 128], F32))
        w_a2 = ctx.enter_context(nc.sbuf_tensor("w_a2_sb", [128, 128], F32))
        w_sza = ctx.enter_context(nc.sbuf_tensor("w_sza_sb", [128, 128], F32))
        biasv = ctx.enter_context(nc.sbuf_tensor("bias_sb", [128, 1], F32))
        biasv0 = ctx.enter_context(nc.sbuf_tensor("bias0_sb", [128, 1], F32))
        sinb = ctx.enter_context(nc.sbuf_tensor("sinb_sb", [64, 1], F32))
        ybuf = [
            ctx.enter_context(nc.sbuf_tensor(f"ybuf{i}", [128, BS * J], F32))
            for i in range(3)
        ]
        x2buf = [
            ctx.enter_context(nc.sbuf_tensor(f"x2buf{i}", [128, BS * J], F32))
            for i in range(2)
        ]
        pbuf = ctx.enter_context(nc.sbuf_tensor("pbuf", [64, RR * J], F32))
        nbuf = ctx.enter_context(nc.sbuf_tensor("nbuf", [64, RR * J], F32))
        wbuf = ctx.enter_context(nc.sbuf_tensor("wbuf", [64, RR * J], F32))
        pb = [
            ctx.enter_context(nc.psum_tensor(f"pb{i}", [128, J], F32))
            for i in range(NBANK)
        ]
        s_upl = ctx.enter_context(nc.semaphore("s_upl"))
        s_zin = ctx.enter_context(nc.semaphore("s_zin"))
        s_pe = ctx.enter_context(nc.semaphore("s_pe"))
        s_dve = ctx.enter_context(nc.semaphore("s_dve"))
        s_pool = ctx.enter_context(nc.semaphore("s_pool"))
        s_m = ctx.enter_context(nc.semaphore("s_m"))
        s_act = ctx.enter_context(nc.semaphore("s_act"))
        s_yout = ctx.enter_context(nc.semaphore("s_yout"))
        block = ctx.enter_context(nc.Block())

        n_upl = 9  # w_a, w_sz, w_a2, w_sza, biasv, biasv0, sinb, y0s, sinit

        @block.sync
        def _(sync):
            sync.dma_start(w_a[:, :], w_a_d[:, :]).then_inc(s_upl, 16)
            sync.dma_start(w_sz[:, :], w_sz_d[:, :]).then_inc(s_upl, 16)
            sync.dma_start(w_a2[:, :], w_a2_d[:, :]).then_inc(s_upl, 16)
            sync.dma_start(w_sza[:, :], w_sza_d[:, :]).then_inc(s_upl, 16)
            sync.dma_start(biasv[:, :], bias_d[:, :]).then_inc(s_upl, 16)
            sync.dma_start(biasv0[:, :], bias0_d[:, :]).then_inc(s_upl, 16)
            sync.dma_start(sinb[:, :], sinb_d[:, :]).then_inc(s_upl, 16)
            sync.dma_start(ybuf[0][:, 0:J], y0s_d[:, :]).then_inc(s_upl, 16)
            sync.dma_start(x2buf[0][0:64, 0 : 2 * J], sinit_d[:, :]).then_inc(
                s_upl, 16
            )
            t1 = min(nt, BS)
            sync.dma_start(
                x2buf[0][64:128, 0 : t1 * J], zin_d[:, 0:t1, :]
            ).then_inc(s_zin, 16)

            for m in range(nblk):
                zm = m + 1
                if zm * BS < nt:
                    t0, t1 = zm * BS, min(nt, zm * BS + BS)
                    if m >= 1:
                        sync.wait_ge(s_pe, m * BS + 1)
                    sync.dma_start(
                        x2buf[zm % 2][64:128, 0 : (t1 - t0) * J],
                        zin_d[:, t0:t1, :],
                    ).then_inc(s_zin, 16)
                if m >= 1:
                    b = m - 1
                    lastslot = min(nslot - 1, b * BS + BS - 1)
                    sync.wait_ge(s_dve, lastslot)
                    pn = min(BS, nslot - 1 - b * BS)
                    if pn > 0:
                        sync.dma_start(
                            ophi_d[:, b * BS + 1 : b * BS + 1 + pn, :],
                            ybuf[b % 3][0:64, 0 : pn * J].rearrange(
                                "p (t j) -> p t j", j=J
                            ),
                        ).then_inc(s_yout, 16)
                    vn = min(BS, nslot - b * BS)
                    sync.dma_start(
                        ov_d[:, b * BS : b * BS + vn, :],
                        ybuf[b % 3][64:128, 0 : vn * J].rearrange(
                            "p (t j) -> p t j", j=J
                        ),
                    ).then_inc(s_yout, 16)
            b = nblk - 1
            sync.wait_ge(s_dve, nt)
            pn = min(BS, nslot - 1 - b * BS)
            n_out = 2 * (nblk - 1) + 1
            if pn > 0:
                n_out += 1
                sync.dma_start(
                    ophi_d[:, b * BS + 1 : b * BS + 1 + pn, :],
                    ybuf[b % 3][0:64, 0 : pn * J].rearrange("p (t j) -> p t j", j=J),
                ).then_inc(s_yout, 16)
            vn = min(BS, nslot - b * BS)
            sync.dma_start(
                ov_d[:, b * BS : b * BS + vn, :],
                ybuf[b % 3][64:128, 0 : vn * J].rearrange("p (t j) -> p t j", j=J),
            ).then_inc(s_yout, 16)
            sync.wait_ge(s_yout, 16 * n_out)

        @block.tensor
        def _(pe):
            pe.wait_ge(s_upl, 16 * n_upl)
            for t in range(nt):
                m = t // BS
                c = (t % BS) * J
                if t % BS == 0:
                    pe.wait_ge(s_zin, 16 * (m + 1))
                if t == 0:
                    pe.matmul(
                        pb[0][:, :], w_sz[:, :], x2buf[0][:, 0:J],
                        start=True, stop=False,
                    )
                    pe.matmul(
                        pb[0][:, :], w_a[:, :], ybuf[0][:, 0:J],
                        start=False, stop=True,
                    ).then_inc(s_pe, 1)
                    continue
                mp_ = (t - 1) // BS
                cp_ = ((t - 1) % BS) * J
                # Y_{t+1} = A^2*Y_{t-1} + (A*SZ)*X2_{t-1} + SZ*X2_t (+ bias')
                pe.wait_ge(s_dve, t - 1 if t >= 2 else 1)
                pe.matmul(
                    pb[t % NBANK][:, :],
                    w_a2[:, :] if t >= 2 else w_a[:, :],
                    ybuf[(mp_ if t >= 2 else m) % 3][:, (cp_ if t >= 2 else c) : (cp_ if t >= 2 else c) + J],
                    start=True,
                    stop=False,
                )
                if t >= 2:
                    pe.matmul(
                        pb[t % NBANK][:, :],
                        w_sza[:, :],
                        x2buf[mp_ % 2][:, cp_ : cp_ + J],
                        start=False,
                        stop=False,
                    )
                if t >= 2:
                    pe.wait_ge(s_act, t - 1)
                pe.matmul(
                    pb[t % NBANK][:, :],
                    w_sz[:, :],
                    x2buf[m % 2][:, c : c + J],
                    start=False,
                    stop=True,
                ).then_inc(s_pe, 1)

        @block.vector
        def _(dve):
            for t in range(nt):
                s = t + 1
                mb = s // BS
                c2 = (s % BS) * J
                if s % BS == 0 and mb >= 3:
                    dve.wait_ge(s_yout, 32 * (mb - 2))
                dve.wait_ge(s_pe, t + 1)
                dve.tensor_scalar(
                    ybuf[mb % 3][:, c2 : c2 + J],
                    pb[t % NBANK][:, :],
                    (biasv0 if t < 2 else biasv)[:, 0:1],
                    None,
                    op0=A.add,
                ).then_inc(s_dve, 1)
                if t < nred:
                    rc = (s % RR) * J
                    rp = ((s - 1) % RR) * J if t >= 1 else rc
                    dve.wait_ge(s_m, max(1, t))
                    if t >= RR - 1:
                        dve.wait_ge(s_act, t - (RR - 2))  # wbuf slot reuse
                    dve.scalar_tensor_tensor(
                        wbuf[:, rc : rc + J],
                        ybuf[mb % 3][0:64, c2 : c2 + J],
                        0.0,
                        nbuf[:, rp : rp + J],
                        op0=A.add,
                        op1=A.subtract,
                    ).then_inc(s_pool, 1)

        @block.gpsimd
        def _(pool):
            # off-critical wrap pipeline: q = phi/2pi + (2^23+256) rounds the
            # turn count; m = (q - (2^23+256)) * 2pi recovers the wrap multiple
            # (exact Sterbenz subtract, then one rounding). DVE consumes the
            # one-step-stale m in its w' = phi - m STT.
            for t in range(nred):
                s = t + 1
                mb = s // BS
                c2 = (s % BS) * J
                rc = (s % RR) * J
                pool.wait_ge(s_dve, t + 1)
                if t >= RR - 1:
                    pool.wait_ge(s_pool, t - (RR - 2))  # nbuf slot reuse
                pool.tensor_scalar(
                    pbuf[:, rc : rc + J],
                    ybuf[mb % 3][0:64, c2 : c2 + J],
                    IC, B1, op0=A.mult, op1=A.add,
                )
                pool.tensor_scalar(
                    nbuf[:, rc : rc + J], pbuf[:, rc : rc + J],
                    -B1, C2PI, op0=A.add, op1=A.mult,
                ).then_inc(s_m, 1)

        @block.scalar
        def _(act):
            for t in range(nred):
                tgt = t + 2
                ma = tgt // BS
                ca = (tgt % BS) * J
                rc = ((t + 1) % RR) * J
                act.wait_ge(s_pool, t + 1)
                act.activation(
                    x2buf[ma % 2][0:64, ca : ca + J],
                    wbuf[:, rc : rc + J],
                    mybir.ActivationFunctionType.Sin,
                    bias=sinb[:, 0:1],
                    scale=1.0,
                ).then_inc(s_act, 1)

    return nc


def _host_prep(params, y0, noise, T, N):
    f32 = np.float32
    nt = noise.shape[0]
    dt = f32(np.float32(T) / np.float32(int(N) - 1))
    d = float(dt)
    sqdt = float(np.sqrt(dt).astype(f32))
    p = np.asarray(params, dtype=np.float64)
    b1, b2, i1, i2, k1, k2, s1, s2 = (float(v) for v in p)

    cv1 = [-d * k1, d * k1, 1 - d * b1 + d * d * k1, -d * d * k1]
    cv2 = [d * k2, -d * k2, -d * d * k2, 1 - d * b2 + d * d * k2]
    cp1 = [1 + d * cv1[0], d * cv1[1], d * cv1[2], d * cv1[3]]
    cp2 = [d * cv2[0], 1 + d * cv2[1], d * cv2[2], d * cv2[3]]
    Am = np.array([cp1, cp2, cv1, cv2], dtype=f32)  # [out, in]
    SZ = np.array(
        [
            [-d * d, 0.0, -d, 0.0],
            [0.0, -d * d, 0.0, -d],
            [d * s1 * sqdt, 0.0, s1 * sqdt, 0.0],
            [0.0, d * s2 * sqdt, 0.0, s2 * sqdt],
        ],
        dtype=f32,
    )  # [in, out]

    Am64 = np.array([cp1, cp2, cv1, cv2], dtype=np.float64)
    SZ64 = SZ.astype(np.float64)
    A2 = (Am64 @ Am64).astype(f32)       # two-step drift
    SZA = (SZ64 @ Am64.T).astype(f32)    # step-(t-1) SZ terms pushed through A

    def expand(M, transpose_in_out):
        w = np.zeros((128, 128), dtype=f32)
        eye_ = np.eye(G, dtype=f32)
        for cin in range(4):
            for cout in range(4):
                coef = M[cout, cin] if transpose_in_out else M[cin, cout]
                w[cin * G : (cin + 1) * G, cout * G : (cout + 1) * G] = coef * eye_
        return w

    w_a = expand(Am, True)
    w_sz = expand(SZ, False)
    w_a2 = expand(A2, True)
    w_sza = expand(SZA, False)
    b4 = np.array([d * d * i1, d * d * i2, d * i1, d * i2], dtype=np.float64)
    b4p = Am64 @ b4 + b4
    biasv0 = np.empty((128, 1), dtype=f32)
    biasv = np.empty((128, 1), dtype=f32)
    for c_ in range(4):
        biasv0[c_ * G : (c_ + 1) * G] = f32(b4[c_])
        biasv[c_ * G : (c_ + 1) * G] = f32(b4p[c_])
    sinb = np.zeros((64, 1), dtype=f32)  # sin bias (w' is already in radians)

    y0 = np.asarray(y0, dtype=f32)
    noise = np.asarray(noise, dtype=f32)
    dtf = f32(dt)

    in_maps = []
    for ci in range(N_CORES):
        yc = y0[ci * BPC : (ci + 1) * BPC].reshape(G, J, 4)
        phi10, v10 = yc[:, :, 0], yc[:, :, 1]
        phi20, v20 = yc[:, :, 2], yc[:, :, 3]
        P1 = (phi10 + v10 * dtf).astype(f32)
        P2 = (phi20 + v20 * dtf).astype(f32)
        y0s = np.concatenate([P1, P2, v10, v20], axis=0)
        sinit = np.empty((64, 2 * J), dtype=f32)
        sinit[0:32, 0:J] = np.sin(phi10, dtype=f32)
        sinit[32:64, 0:J] = np.sin(phi20, dtype=f32)
        sinit[0:32, J : 2 * J] = np.sin(P1, dtype=f32)
        sinit[32:64, J : 2 * J] = np.sin(P2, dtype=f32)
        zc_block = noise[:, ci * BPC : (ci + 1) * BPC, :].reshape(nt, G, J, 2)
        zin = np.ascontiguousarray(zc_block.transpose(3, 1, 0, 2)).reshape(64, nt, J)
        in_maps.append(
            {
                "w_a": np.ascontiguousarray(w_a),
                "w_sz": np.ascontiguousarray(w_sz),
                "w_a2": np.ascontiguousarray(w_a2),
                "w_sza": np.ascontiguousarray(w_sza),
                "biasv": biasv,
                "biasv0": biasv0,
                "sinb": sinb,
                "y0s": np.ascontiguousarray(y0s),
                "sinit": sinit,
                "zin": zin,
            }
        )
    return in_maps


def run_device(params, y0, noise, T, N, trace=False):
    nt = noise.shape[0]
    key = nt
    if key not in _CACHE:
        _CACHE[key] = _build_program(nt)
    nc = _CACHE[key]
    in_maps = _host_prep(params, y0, noise, T, N)
    res = bass_utils.run_bass_kernel_spmd(
        nc, in_maps, core_ids=list(range(N_CORES)), trace=trace
    )
    nslot = nt + 1
    traj = np.empty((BATCH, nslot, 4), dtype=np.float32)
    for ci in range(N_CORES):
        phi = res.results[ci]["out_phi"].reshape(2, G, nslot, J)
        v = res.results[ci]["out_v"].reshape(2, G, nslot, J)
        sl = slice(ci * BPC, (ci + 1) * BPC)
        traj[sl, :, 0] = phi[0].transpose(0, 2, 1).reshape(BPC, nslot)
        traj[sl, :, 2] = phi[1].transpose(0, 2, 1).reshape(BPC, nslot)
        traj[sl, :, 1] = v[0].transpose(0, 2, 1).reshape(BPC, nslot)
        traj[sl, :, 3] = v[1].transpose(0, 2, 1).reshape(BPC, nslot)
    traj[:, 0, :] = np.asarray(y0, dtype=np.float32)
    return res, traj


def kernel(params, y0, noise, T, N):
    res, traj = run_device(params, y0, noise, int(T), int(N))
    return traj
